# revision 1
# baseline (speedup 1.0000x reference)
"""MultiHeadCrossAttention kernel for 8 Trainium2 NeuronCores.

Sharding: pure data-parallel over batch (B=8 -> 1 batch element per core).
Per-core layout strategy:
  - Activations transposed on-chip via PE transpose -> feature-major xT/keyT/valueT.
  - Projections produce qT,kT feature-major [E, L] and v token-major [L, E]
    (v stored with a ones-column per head for the softmax denominator).
  - Attention per head in transposed orientation: scoresT[k,q] = kT_h^T-slices,
    exp on ScalarE (no max subtraction: |scores*0.125| < ~4), attn_unnormT and
    denominator from one matmul using the [v_h | 1] stationary operand.
  - attn_weights (mean over heads of normalized probs) accumulated in PSUM via
    identity matmuls, transposed back to natural [q,k] at the end of each
    q-block with PE transposes.
  - out_proj + residual + LayerNorm fused per q-block of 256 rows.
All matmuls run as float32r (full PE rate at free-dim >= 256).
"""

import numpy as np
from contextlib import ExitStack

import concourse.bacc as bacc
import concourse.bass as bass
import concourse.tile as tile
from concourse import mybir
from concourse.bass_utils import run_bass_kernel_spmd
from concourse.masks import make_identity

E = 1024
H = 16
DH = 64
L = 1024
P = 128
QB = 256          # q-block size
NQB = L // QB     # 4
NKT = L // P      # 8 k-tiles
NEC = E // P      # 8 feature chunks
VS = H * (DH + 1)  # 1040 v columns per k-chunk (65 per head)
LN_EPS = 1e-5

F32 = mybir.dt.float32
F32R = mybir.dt.float32r
AF = mybir.ActivationFunctionType
OP = mybir.AluOpType


def _emit(nc, tc, io):
    x_q, k_in, v_in = io["x_q"], io["k_in"], io["v_in"]
    wqT, wkT, wvT, woT_d = io["wqT"], io["wkT"], io["wvT"], io["woT"]
    b_all, gb = io["b_all"], io["gb"]
    y_out, w_out = io["y_out"], io["w_out"]

    ctx = tc.ctx  # ExitStack from caller
    ctx.enter_context(nc.allow_low_precision("fp32r tiles"))

    const = ctx.enter_context(tc.tile_pool(name="const", bufs=1))
    persist = ctx.enter_context(tc.tile_pool(name="persist", bufs=1))
    psum_acc = ctx.enter_context(tc.tile_pool(name="psum_acc", bufs=4, space="PSUM"))
    psum_sc = ctx.enter_context(tc.tile_pool(name="psum_sc", bufs=2, space="PSUM"))
    psum_av = ctx.enter_context(tc.tile_pool(name="psum_av", bufs=2, space="PSUM"))

    ident_f = const.tile([P, P], F32)
    make_identity(nc, ident_f[:])
    ident = const.tile([P, P], F32R)
    nc.vector.tensor_copy(ident[:], ident_f[:])
    ones1f = const.tile([1, P], F32)
    nc.vector.memset(ones1f[:], 1.0)
    ones1 = const.tile([1, P], F32R)
    nc.vector.tensor_copy(ones1[:], ones1f[:])
    onesP = const.tile([P, 1], F32)
    nc.vector.memset(onesP[:], 1.0)
    eps_sb = const.tile([P, 1], F32)
    nc.vector.memset(eps_sb[:], LN_EPS)

    # biases: b_all DRAM [4, 1024] rows = bq, bk, bv, bo ; gb DRAM [2, 1024] = gamma, beta
    bqk_col = const.tile([P, 2 * NEC], F32)  # [:,0:8]=bq cols, [:,8:16]=bk cols
    for i in range(2):
        nc.sync.dma_start(
            out=bqk_col[:, NEC * i:NEC * (i + 1)],
            in_=b_all[i, :].rearrange("(m p) -> p m", p=P).bitcast(F32),
        )
    bvbo_row = const.tile([1, 2 * E], F32R)  # [0:1024]=bv, [1024:2048]=bo
    nc.sync.dma_start(out=bvbo_row[:, 0:E], in_=b_all[2:3, :])
    nc.sync.dma_start(out=bvbo_row[:, E:2 * E], in_=b_all[3:4, :])
    gb_row = const.tile([1, 2 * E], F32R)
    nc.sync.dma_start(out=gb_row[:], in_=gb[:])

    qT = persist.tile([P, NEC * L], F32R)      # [e_out, l] chunks of 128 rows
    kT = persist.tile([P, NEC * L], F32R)
    v_sb = persist.tile([P, NKT * VS], F32R)   # token-major v, 65-wide head slots
    op_ = onesP[:]
    nc.vector.tensor_copy(
        out=v_sb[:].rearrange("p (n d) -> p n d", d=DH + 1)[:, :, DH:DH + 1],
        in_=bass.AP(tensor=op_.tensor, offset=op_.offset,
                    ap=[op_.ap[0], [0, H * NKT], [0, 1]]),
    )

    # ---------------- phase 1: transposes + projections ----------------
    with tc.tile_pool(name="wt", bufs=1) as wt_pool, \
         tc.tile_pool(name="ld", bufs=3) as ld_pool, \
         tc.tile_pool(name="actT", bufs=1) as actT_pool:

        for ti, (src, w_d) in enumerate([(x_q, wqT), (k_in, wkT), (v_in, wvT)]):
            # transposed activation aT [e_in, l]
            aT = actT_pool.tile([P, NEC * L], F32R, tag="actT")
            for lc in range(NKT):
                nat = ld_pool.tile([P, E], F32R, tag="ld")
                nc.sync.dma_start(out=nat[:], in_=src[P * lc:P * (lc + 1), :])
                for ep in range(NEC // 2):
                    tp = psum_av.tile([P, 2 * P], F32R, tag="av", name=f"tp_{ti}_{lc}_{ep}")
                    for sub in range(2):
                        ec = 2 * ep + sub
                        nc.tensor.transpose(
                            tp[:, P * sub:P * (sub + 1)],
                            nat[:, P * ec:P * (ec + 1)], ident[:],
                        )
                    for sub in range(2):
                        ec = 2 * ep + sub
                        dst = aT[:, L * ec + P * lc: L * ec + P * lc + P]
                        if (lc + ep) % 2 == 0:
                            nc.scalar.copy(dst, tp[:, P * sub:P * (sub + 1)])
                        else:
                            nc.vector.tensor_copy(dst, tp[:, P * sub:P * (sub + 1)])
            wt = wt_pool.tile([P, NEC * E], F32R, tag="wt")
            for c in range(NEC):
                nc.sync.dma_start(
                    out=wt[:, E * c:E * (c + 1)], in_=w_d[P * c:P * (c + 1), :]
                )
            tiles16 = [(m, n) for m in range(NEC) for n in range(2)]
            for g in range(0, 16, 4):
                grp = tiles16[g:g + 4]
                psums = [
                    psum_acc.tile([P, 512], F32, tag="acc", name=f"ps_{ti}_{g}_{i}")
                    for i in range(len(grp))
                ]
                for c in range(NEC):
                    for i, (m, n) in enumerate(grp):
                        if ti < 2:  # qT / kT : feature-major out
                            lhsT = wt[:, E * c + P * m: E * c + P * (m + 1)]
                            rhs = aT[:, L * c + 512 * n: L * c + 512 * (n + 1)]
                        else:       # v natural
                            lhsT = aT[:, L * c + P * m: L * c + P * (m + 1)]
                            rhs = wt[:, E * c + 512 * n: E * c + 512 * (n + 1)]
                        nc.tensor.matmul(
                            psums[i][:], lhsT, rhs,
                            start=(c == 0), stop=(c == NEC - 1 and ti < 2),
                        )
                for i, (m, n) in enumerate(grp):
                    if ti < 2:
                        dst = (qT if ti == 0 else kT)[:, L * m + 512 * n: L * m + 512 * (n + 1)]
                        nc.vector.tensor_scalar_add(
                            out=dst, in0=psums[i][:],
                            scalar1=bqk_col[:, NEC * ti + m: NEC * ti + m + 1],
                        )
                    else:
                        # bias via ones-row matmul, then strided evict into head slots
                        nc.tensor.matmul(
                            psums[i][:], ones1[0:1, :],
                            bvbo_row[0:1, 512 * n:512 * (n + 1)],
                            start=False, stop=True,
                        )
                        dst = v_sb[:, VS * m + 520 * n: VS * m + 520 * (n + 1)]
                        nc.vector.tensor_copy(
                            out=dst.rearrange("p (h d) -> p h d", d=DH + 1)[:, :, 0:DH],
                            in_=psums[i][:].rearrange("p (h d) -> p h d", d=DH),
                        )

    # ---------------- phase 2: attention + out_proj + LN ----------------
    with tc.tile_pool(name="wo", bufs=1) as wo_pool, \
         tc.tile_pool(name="expT", bufs=2) as expT_pool, \
         tc.tile_pool(name="attnT", bufs=1) as attnT_pool, \
         tc.tile_pool(name="invbc", bufs=2) as invbc_pool, \
         tc.tile_pool(name="accq", bufs=1) as accq_pool, \
         tc.tile_pool(name="wnat", bufs=4) as wnat_pool, \
         tc.tile_pool(name="xqb", bufs=1) as xqb_pool, \
         tc.tile_pool(name="ysb", bufs=1) as ysb_pool, \
         tc.tile_pool(name="small", bufs=2) as small:

        woT = wo_pool.tile([P, NEC * E], F32R, tag="wo")
        for c in range(NEC):
            nc.sync.dma_start(out=woT[:, E * c:E * (c + 1)], in_=woT_d[P * c:P * (c + 1), :])
        gamma_bc = wo_pool.tile([P, E], mybir.dt.bfloat16, tag="gbc")
        beta_bc = wo_pool.tile([P, E], mybir.dt.bfloat16, tag="bbc")
        for i, dstt in enumerate([gamma_bc, beta_bc]):
            for hf in range(2):
                bcp = psum_sc.tile([P, 512], F32, tag="sc")
                nc.tensor.matmul(
                    bcp[:], ones1[0:1, :],
                    gb_row[0:1, E * i + 512 * hf: E * i + 512 * (hf + 1)],
                    start=True, stop=True,
                )
                nc.scalar.copy(dstt[:, 512 * hf:512 * (hf + 1)], bcp[:])

        for qb in range(NQB):
            q0 = QB * qb
            attnT = attnT_pool.tile([P, NEC * QB], F32R, tag="attnT")
            accs = [
                psum_acc.tile([P, 512], F32, tag="acc", name=f"acc_{qb}_{j}")
                for j in range(4)
            ]
            def head_front(h):
                hb = (h % 2) * DH
                hc = h // 2
                expT = expT_pool.tile(
                    [P, NKT * QB], F32R, tag="expT", name=f"expT_{qb}_{h}"
                )
                for j in range(4):
                    sc = psum_sc.tile([P, 512], F32, tag="sc", name=f"sc_{qb}_{h}_{j}")
                    for half in range(2):
                        kt = 2 * j + half
                        lhsT = kT[hb:hb + DH, L * hc + P * kt: L * hc + P * (kt + 1)]
                        rhs = qT[hb:hb + DH, L * hc + q0: L * hc + q0 + QB]
                        nc.tensor.matmul(
                            sc[:, QB * half:QB * (half + 1)],
                            lhsT, rhs,
                            start=True, stop=True,
                        )
                    nc.scalar.activation(
                        expT[:, 512 * j:512 * (j + 1)], sc[:], AF.Exp, scale=0.125
                    )
                return expT

            def head_tail(h, expT):
                hb = (h % 2) * DH
                hc = h // 2
                av = psum_av.tile([DH + 1, QB], F32, tag="av", name=f"av_{qb}_{h}")
                for kt in range(NKT):
                    nc.tensor.matmul(
                        av[:],
                        v_sb[:, VS * kt + (DH + 1) * h: VS * kt + (DH + 1) * (h + 1)],
                        expT[:, QB * kt:QB * (kt + 1)],
                        start=(kt == 0), stop=(kt == NKT - 1),
                    )
                inv = small.tile([1, QB], F32R, tag="inv", name=f"inv_{qb}_{h}")
                nc.vector.reciprocal(inv[:], av[DH:DH + 1, :])
                bcp = psum_sc.tile([P, QB], F32, tag="sc", name=f"bcp_{qb}_{h}")
                nc.tensor.matmul(
                    bcp[:], ones1[0:1, :], inv[:],
                    start=True, stop=True,
                )
                inv_bc = invbc_pool.tile([P, QB], F32, tag="invbc", name=f"ib_{qb}_{h}")
                nc.scalar.copy(inv_bc[:], bcp[:])
                nc.vector.tensor_tensor(
                    out=attnT[hb:hb + DH, QB * hc:QB * (hc + 1)],
                    in0=av[0:DH, :], in1=inv_bc[0:DH, :], op=OP.mult,
                )
                iap = inv_bc[:]
                bc_ap = bass.AP(
                    tensor=iap.tensor, offset=iap.offset,
                    ap=[iap.ap[0], [0, NKT], iap.ap[1]],
                )
                nc.vector.tensor_tensor(
                    out=expT[:].rearrange("p (n d) -> p n d", d=QB),
                    in0=expT[:].rearrange("p (n d) -> p n d", d=QB),
                    in1=bc_ap, op=OP.mult,
                )
                for j in range(4):
                    nc.tensor.matmul(
                        accs[j][:],
                        ident[:],
                        expT[:, 512 * j:512 * (j + 1)],
                        start=(h == 0), stop=(h == H - 1),
                    )

            for h in range(H):
                head_tail(h, head_front(h))
            # attn_weights: evict acc (mean over heads), transpose to natural
            accq = accq_pool.tile([P, NKT * QB], F32R, tag="accq")
            for j in range(4):
                nc.scalar.mul(accq[:, 512 * j:512 * (j + 1)], accs[j][:], 1.0 / H)
            for kt in range(NKT):
                for qs in range(2):
                    tp = psum_av.tile([P, P], F32R, tag="av")
                    nc.tensor.transpose(
                        tp[:], accq[:, QB * kt + P * qs: QB * kt + P * (qs + 1)], ident[:]
                    )
                    wb = wnat_pool.tile([P, P], F32, tag="wnat", name=f"wb_{qb}_{kt}_{qs}")
                    nc.vector.tensor_copy(out=wb[:], in_=tp[:])
                    nc.sync.dma_start(
                        out=w_out[q0 + P * qs: q0 + P * (qs + 1), P * kt:P * (kt + 1)],
                        in_=wb[:],
                    )
            # out_proj + residual + LN
            x_qb = xqb_pool.tile([P, 2 * E], F32R, tag="xqb")
            for qs in range(2):
                nc.sync.dma_start(
                    out=x_qb[:, E * qs:E * (qs + 1)],
                    in_=x_q[q0 + P * qs: q0 + P * (qs + 1), :],
                )
            y_sb = ysb_pool.tile([P, 2 * E], F32, tag="ysb")
            for qs in range(2):
                for eb in range(2):
                    po = psum_acc.tile([P, 512], F32, tag="acc")
                    for c in range(NEC):
                        nc.tensor.matmul(
                            po[:],
                            attnT[:, QB * c + P * qs: QB * c + P * (qs + 1)],
                            woT[:, E * c + 512 * eb: E * c + 512 * (eb + 1)],
                            start=(c == 0), stop=False,
                        )
                    nc.tensor.matmul(
                        po[:], ones1[0:1, :],
                        bvbo_row[0:1, E + 512 * eb: E + 512 * (eb + 1)],
                        start=False, stop=True,
                    )
                    nc.vector.tensor_tensor(
                        out=y_sb[:, E * qs + 512 * eb: E * qs + 512 * (eb + 1)],
                        in0=po[:], in1=x_qb[:, E * qs + 512 * eb: E * qs + 512 * (eb + 1)],
                        op=OP.add,
                    )
                ych = y_sb[:, E * qs:E * (qs + 1)]
                stats = small.tile([P, 2, 6], F32, tag="stats")
                ychg = ych.rearrange("p (s f) -> p s f", f=512)
                for sg in range(2):
                    nc.vector.bn_stats(out=stats[:, sg, :], in_=ychg[:, sg, :])
                mv = small.tile([P, 2], F32, tag="mv")
                nc.vector.bn_aggr(out=mv[:], in_=stats[:])
                std = small.tile([P, 1], F32, tag="std")
                nc.scalar.activation(std[:], mv[:, 1:2], AF.Sqrt, bias=eps_sb[:])
                rstd = small.tile([P, 1], F32, tag="rstd")
                nc.vector.reciprocal(rstd[:], std[:])
                nc.vector.tensor_scalar(
                    out=ych, in0=ych, scalar1=mv[:, 0:1], scalar2=rstd[:],
                    op0=OP.subtract, op1=OP.mult,
                )
                nc.vector.tensor_tensor(out=ych, in0=ych, in1=gamma_bc[:], op=OP.mult)
                nc.vector.tensor_tensor(out=ych, in0=ych, in1=beta_bc[:], op=OP.add)
                nc.sync.dma_start(
                    out=y_out[q0 + P * qs: q0 + P * (qs + 1), :], in_=ych
                )


_CACHED = None


def _build():
    global _CACHED
    if _CACHED is not None:
        return _CACHED
    nc = bacc.Bacc("TRN2", target_bir_lowering=False, debug=False, num_devices=8)
    io = {}
    for name in ["x_q", "k_in", "v_in", "wqT", "wkT", "wvT", "woT"]:
        io[name] = nc.dram_tensor(name, [1024, 1024], F32R, kind="ExternalInput").ap()
    io["b_all"] = nc.dram_tensor("b_all", [4, 1024], F32R, kind="ExternalInput").ap()
    io["gb"] = nc.dram_tensor("gb", [2, 1024], F32R, kind="ExternalInput").ap()
    io["y_out"] = nc.dram_tensor("y_out", [1024, 1024], F32, kind="ExternalOutput").ap()
    io["w_out"] = nc.dram_tensor("w_out", [1024, 1024], F32, kind="ExternalOutput").ap()
    with tile.TileContext(nc) as tc:
        with ExitStack() as ctx:
            tc.ctx = ctx
            _emit(nc, tc, io)
    nc.compile()
    _CACHED = nc
    return nc


def kernel(query, key_t, value, in_proj_w, in_proj_b, out_proj_w, out_proj_b,
           ln_gamma, ln_beta, _trace=False, _tmpdir=None):
    query = np.ascontiguousarray(np.asarray(query, dtype=np.float32))
    key_t = np.ascontiguousarray(np.asarray(key_t, dtype=np.float32))
    value = np.ascontiguousarray(np.asarray(value, dtype=np.float32))
    in_proj_w = np.asarray(in_proj_w, dtype=np.float32)
    wqT = np.ascontiguousarray(in_proj_w[0:E].T)
    wkT = np.ascontiguousarray(in_proj_w[E:2 * E].T)
    wvT = np.ascontiguousarray(in_proj_w[2 * E:3 * E].T)
    woT = np.ascontiguousarray(np.asarray(out_proj_w, dtype=np.float32).T)
    b = np.asarray(in_proj_b, dtype=np.float32)
    b_all = np.ascontiguousarray(
        np.stack([b[0:E], b[E:2 * E], b[2 * E:3 * E],
                  np.asarray(out_proj_b, dtype=np.float32)])
    )
    gb = np.ascontiguousarray(
        np.stack([np.asarray(ln_gamma, dtype=np.float32),
                  np.asarray(ln_beta, dtype=np.float32)])
    )
    nc = _build()
    in_maps = [
        dict(x_q=query[c], k_in=key_t[c], v_in=value[c],
             wqT=wqT, wkT=wkT, wvT=wvT, woT=woT, b_all=b_all, gb=gb)
        for c in range(8)
    ]
    res = run_bass_kernel_spmd(
        nc, in_maps, core_ids=list(range(8)), trace=_trace, tmpdir=_tmpdir
    )
    y = np.stack([r["y_out"] for r in res.results])
    w = np.stack([r["w_out"] for r in res.results])
    kernel._last_result = res
    return y, w



# revision 11
# speedup vs baseline: 1.4187x; 1.4187x over previous
"""MultiHeadCrossAttention kernel for 8 Trainium2 NeuronCores (v3).

Sharding: pure data-parallel over batch (B=8 -> 1 batch element per core).

Strategy:
  - Host pre-transposes activations and casts everything to bf16; no
    on-chip activation transposes.
  - Dtypes chosen around two hardware rules: matmul operands must share
    a dtype family (no f32r x bf16), and every matmul with a non-f32
    moving operand is split into Ldweights+Matmult by legalization
    (2 PE instructions). Scores run in f32r (moving free-dim 256 -> full
    rate, single instruction); projections / attnV / identity / out_proj
    run in bf16 (full rate at any size).
  - QB=256 q-blocks; per (head, q-block): scores into 4 ping-pong
    [128,512] PSUM tiles (2 banks), exp'd to a bf16 expT tile; attnV
    with the [v_h | 1] ones-column denominator trick; probs normalized
    in place on DVE (bf16 2x mode); head-mean attn_weights accumulated
    with a (1/16)-scaled-identity matmul into a 4-bank fp32 PSUM slab,
    evicted by Act and DMA'd K-MAJOR (host transposes with a numpy view).
  - PSUM budget: scores 2 + av/broadcast 2 + weight-slab 4 = 8 banks.
  - LayerNorm rstd = exp(-0.5*ln(var+eps)): Act only ever needs the
    natural_log_exp_and_others table set (one load, no reloads).
  - Software pipelining: per iteration PE runs
    [scores_h | attnV_{h-1} | identity_{h-2} | bcast_{h-1}], draining
    the identity lag at q-block boundaries.
"""

import numpy as np
import ml_dtypes
from contextlib import ExitStack

import concourse.bacc as bacc
import concourse.bass as bass
import concourse.tile as tile
from concourse import mybir
from concourse.bass_utils import run_bass_kernel_spmd
from concourse.masks import make_identity

E = 1024
H = 16
DH = 64
L = 1024
P = 128
QB = 256          # q-block size
NQB = L // QB     # 4
NKT = L // P      # 8 k-tiles
NEC = E // P      # 8 feature chunks
VS = H * (DH + 1)  # 1040 v columns per k-chunk (65 per head)
LN_EPS = 1e-5

F32 = mybir.dt.float32
F32R = mybir.dt.float32r
BF16 = mybir.dt.bfloat16
AF = mybir.ActivationFunctionType
OP = mybir.AluOpType
BF16NP = ml_dtypes.bfloat16
# heads whose probs-normalize multiply runs on the Pool engine (DVE offload)
POOL_MULT_HEADS = frozenset()


def _emit(nc, tc, io):
    qT_in, kT_in, vT_in = io["qT_in"], io["kT_in"], io["vT_in"]
    x_q = io["x_q"]
    w_d = {"q": io["wq"], "k": io["wk"], "v": io["wv"], "o": io["wo"]}
    b_all, gb = io["b_all"], io["gb"]
    y_out, wT_out = io["y_out"], io["wT_out"]

    ctx = tc.ctx
    ctx.enter_context(nc.allow_low_precision("bf16 kernel"))

    const = ctx.enter_context(tc.tile_pool(name="const", bufs=1))
    persist = ctx.enter_context(tc.tile_pool(name="persist", bufs=1))

    # ---- constants ----
    ones1 = const.tile([1, P], BF16)
    nc.vector.memset(ones1[:], 1.0)
    ident_f = const.tile([P, P], F32)
    make_identity(nc, ident_f[:])
    identS = const.tile([P, P], BF16)   # identity / H for head-mean accum
    nc.vector.tensor_scalar_mul(out=identS[:], in0=ident_f[:], scalar1=1.0 / H)
    eps_sb = const.tile([P, 1], F32)
    nc.vector.memset(eps_sb[:], LN_EPS)

    # biases: b_all [4,1024] rows = bq,bk,bv,bo ; gb [2,1024] = gamma,beta
    bqk_col = const.tile([P, 2 * NEC], F32)  # [:,0:8]=bq cols, [:,8:16]=bk
    for i in range(2):
        nc.sync.dma_start(
            out=bqk_col[:, NEC * i:NEC * (i + 1)],
            in_=b_all[i, :].rearrange("(m p) -> p m", p=P),
        )
    bvbo_row = const.tile([1, 2 * E], BF16)  # [0:1024]=bv, [1024:2048]=bo
    gb_row = const.tile([1, 2 * E], BF16)
    with tc.tile_pool(name="tmprow", bufs=1) as tmprow:
        gbvo_f = tmprow.tile([1, 4 * E], F32)
        nc.sync.dma_start(out=gbvo_f[:, 0:E], in_=b_all[2:3, :])
        nc.sync.dma_start(out=gbvo_f[:, E:2 * E], in_=b_all[3:4, :])
        nc.sync.dma_start(out=gbvo_f[:, 2 * E:3 * E], in_=gb[0:1, :])
        nc.sync.dma_start(out=gbvo_f[:, 3 * E:4 * E], in_=gb[1:2, :])
        nc.vector.tensor_copy(bvbo_row[:], gbvo_f[:, 0:2 * E])
        nc.vector.tensor_copy(gb_row[:], gbvo_f[:, 2 * E:4 * E])

    qT_sb = persist.tile([P, NEC * L], F32R)   # [e_out chunks, l] (scores)
    kT_sb = persist.tile([P, NEC * L], F32R)
    v_sb = persist.tile([P, NKT * VS], BF16)   # token-major v, 65-wide slots
    woT_sb = persist.tile([P, NEC * E], BF16)  # [e_in chunks, e_out]

    # ones columns of v_sb (denominator trick): offset DH, stride DH+1
    vap = v_sb[:]
    nc.vector.memset(
        bass.AP(tensor=vap.tensor, offset=vap.offset + DH,
                ap=[vap.ap[0], [DH + 1, H * NKT]]),
        1.0,
    )

    # broadcast gamma/beta to [P, E] via ones-matmul
    gamma_bc = const.tile([P, E], BF16)
    beta_bc = const.tile([P, E], BF16)
    with tc.tile_pool(name="gbp", bufs=2, space="PSUM") as gbp:
        for i, dstt in enumerate([gamma_bc, beta_bc]):
            for hf in range(2):
                bcp = gbp.tile([P, E // 2], F32, tag="gb")
                nc.tensor.matmul(
                    bcp[:], ones1[0:1, :],
                    gb_row[0:1, E * i + 512 * hf: E * i + 512 * (hf + 1)],
                    start=True, stop=True,
                )
                nc.scalar.copy(dstt[:, 512 * hf:512 * (hf + 1)], bcp[:])

    # ---------------- phase 1: projections (bf16 matmuls) ----------------
    with tc.tile_pool(name="ld", bufs=2) as ld_pool, \
         tc.tile_pool(name="wt", bufs=2) as wt_pool, \
         tc.tile_pool(name="p1ps", bufs=6, space="PSUM") as p1ps:

        for ti, (key, src) in enumerate([("q", qT_in), ("k", kT_in), ("v", vT_in)]):
            aT = ld_pool.tile([P, NEC * L], BF16, tag="ld", name=f"aT_{key}")
            nc.sync.dma_start(
                out=aT[:].rearrange("p (c e) -> p c e", e=L),
                in_=src[:].rearrange("(c p) e -> p c e", p=P))
            wt = wt_pool.tile([P, NEC * E], BF16, tag="wt", name=f"w_{key}")
            nc.sync.dma_start(
                out=wt[:].rearrange("p (c e) -> p c e", e=E),
                in_=w_d[key][:].rearrange("(c p) e -> p c e", p=P))

            if ti < 2:
                # qT/kT: out[e_out, l] = sum_c wt[c][:,m]^T @ aT[c][:, l]
                dst = qT_sb if ti == 0 else kT_sb
                for m in range(NEC):
                    for n in range(2):
                        ps = p1ps.tile([P, 512], F32, tag="p1", name=f"p_{key}_{m}_{n}")
                        for c in range(NEC):
                            nc.tensor.matmul(
                                ps[:],
                                wt[:, E * c + P * m: E * c + P * (m + 1)],
                                aT[:, L * c + 512 * n: L * c + 512 * (n + 1)],
                                start=(c == 0), stop=(c == NEC - 1),
                            )
                        if ti == 0:
                            nc.vector.tensor_scalar_add(
                                out=dst[:, L * m + 512 * n: L * m + 512 * (n + 1)],
                                in0=ps[:],
                                scalar1=bqk_col[:, NEC * ti + m: NEC * ti + m + 1],
                            )
                        else:
                            nc.scalar.activation(
                                dst[:, L * m + 512 * n: L * m + 512 * (n + 1)],
                                ps[:], AF.Identity,
                                bias=bqk_col[:, NEC * ti + m: NEC * ti + m + 1],
                            )
            else:
                # v: out[l, e_out] = sum_c aT[c][:, l-tile]^T @ wt[c][:, e]
                for m in range(NEC):
                    for n in range(2):
                        ps = p1ps.tile([P, 512], F32, tag="p1", name=f"p_v_{m}_{n}")
                        for c in range(NEC):
                            nc.tensor.matmul(
                                ps[:],
                                aT[:, L * c + P * m: L * c + P * (m + 1)],
                                wt[:, E * c + 512 * n: E * c + 512 * (n + 1)],
                                start=(c == 0), stop=False,
                            )
                        nc.tensor.matmul(
                            ps[:], ones1[0:1, :],
                            bvbo_row[0:1, 512 * n:512 * (n + 1)],
                            start=False, stop=True,
                        )
                        dst = v_sb[:, VS * m + 520 * n: VS * m + 520 * (n + 1)]
                        nc.vector.tensor_copy(
                            out=dst.rearrange("p (h d) -> p h d", d=DH + 1)[:, :, 0:DH],
                            in_=ps[:].rearrange("p (h d) -> p h d", d=DH),
                        )
        nc.sync.dma_start(
            out=woT_sb[:].rearrange("p (c e) -> p c e", e=E),
            in_=w_d["o"][:].rearrange("(c p) e -> p c e", p=P))

    # ---------------- phase 2: attention + out_proj + LN ----------------
    with tc.tile_pool(name="scps", bufs=2, space="PSUM") as scps, \
         tc.tile_pool(name="avps", bufs=2, space="PSUM") as avps, \
         tc.tile_pool(name="wacc", bufs=1, space="PSUM") as waccp, \
         tc.tile_pool(name="expp", bufs=4) as expp, \
         tc.tile_pool(name="attnT", bufs=2) as attnT_pool, \
         tc.tile_pool(name="xq", bufs=2) as xq_pool, \
         tc.tile_pool(name="ysb", bufs=3) as ysb_pool, \
         tc.tile_pool(name="wsb", bufs=2) as wsb_pool, \
         tc.tile_pool(name="small", bufs=4) as small:

        state = {}

        def emit_scores(qb, h):
            hb = (h % 2) * DH
            hc = h // 2
            q0 = QB * qb
            expT = expp.tile([P, NKT * QB], BF16, tag="expT", name=f"expT_{qb}_{h}")
            for j in range(4):
                sc = scps.tile([P, 512], F32, tag="sc", name=f"sc_{qb}_{h}_{j}")
                for half in range(2):
                    kt = 2 * j + half
                    nc.tensor.matmul(
                        sc[:, 256 * half:256 * (half + 1)],
                        kT_sb[hb:hb + DH, L * hc + P * kt: L * hc + P * (kt + 1)],
                        qT_sb[hb:hb + DH, L * hc + q0: L * hc + q0 + QB],
                        start=True, stop=True, skip_group_check=True,
                    )
                nc.scalar.activation(
                    expT[:, 512 * j:512 * (j + 1)], sc[:], AF.Exp, scale=0.125)
            state[(qb, h)] = expT

        def emit_av(qb, h):
            # attnV + denominators, then reciprocal of the denominator row
            expT = state[(qb, h)]
            avb = avps.tile([P, 512], F32, tag="av", name=f"av_{qb}_{h}")
            for kt in range(NKT):
                nc.tensor.matmul(
                    avb[0:DH + 1, 0:QB],
                    v_sb[:, VS * kt + (DH + 1) * h: VS * kt + (DH + 1) * (h + 1)],
                    expT[:, QB * kt:QB * (kt + 1)],
                    start=(kt == 0), stop=(kt == NKT - 1),
                    skip_group_check=True,
                )
            inv_row = small.tile([1, QB], BF16, tag="inv", name=f"inv_{qb}_{h}")
            nc.vector.reciprocal(inv_row[:], avb[DH:DH + 1, 0:QB])
            state[(qb, h)] = (expT, avb, inv_row)

        def emit_bcast(qb, h):
            expT, avb, inv_row = state[(qb, h)]
            nc.tensor.matmul(
                avb[:, 256:256 + QB], ones1[0:1, :], inv_row[0:1, :],
                start=True, stop=True, skip_group_check=True,
            )
            inv_bc = small.tile([P, QB], BF16, tag="invbc", name=f"ib_{qb}_{h}")
            nc.vector.tensor_copy(inv_bc[:], avb[:, 256:256 + QB])
            # normalize probs in place (bf16, 2x mode)
            iap = inv_bc[:]
            bc_ap = bass.AP(tensor=iap.tensor, offset=iap.offset,
                            ap=[iap.ap[0], [0, NKT], iap.ap[1]])
            eng = nc.gpsimd if h in POOL_MULT_HEADS else nc.vector
            eng.tensor_tensor(
                out=expT[:].rearrange("p (n d) -> p n d", d=QB),
                in0=expT[:].rearrange("p (n d) -> p n d", d=QB),
                in1=bc_ap, op=OP.mult,
            )
            # attn head output column block: av * inv
            hb = (h % 2) * DH
            hc = h // 2
            attnT = state[("attnT", qb)]
            nc.vector.tensor_tensor(
                out=attnT[hb:hb + DH, QB * hc:QB * (hc + 1)],
                in0=avb[0:DH, 0:QB], in1=inv_bc[0:DH, :], op=OP.mult,
            )

        def emit_identity(qb, h):
            expT = state.pop((qb, h))[0]
            wacc = state[("wacc", qb)]
            for j in range(4):
                nc.tensor.matmul(
                    wacc[:, 512 * j:512 * (j + 1)],
                    identS[:],
                    expT[:, 512 * j:512 * (j + 1)],
                    start=(h == 0), stop=(h == H - 1),
                    skip_group_check=True,
                )

        def emit_qb_head(qb):
            state[("attnT", qb)] = attnT_pool.tile(
                [P, NEC * QB], BF16, tag="attnT", name=f"attnT_{qb}")
            state[("wacc", qb)] = waccp.tile(
                [P, NKT * QB], F32, tag="wacc", name=f"wacc_{qb}")
            x_sb = xq_pool.tile([P, 2 * E], BF16, tag="xq", name=f"x_{qb}")
            nc.sync.dma_start(
                out=x_sb[:].rearrange("p (s e) -> p s e", e=E),
                in_=x_q[QB * qb:QB * (qb + 1), :].rearrange("(s p) e -> p s e", p=P))
            state[("x", qb)] = x_sb

        def emit_qb_tail(qb):
            q0 = QB * qb
            # ---- attn weights evict + DMA (K-major; host transposes) ----
            wacc = state.pop(("wacc", qb))
            wsb = wsb_pool.tile([P, NKT * QB], BF16, tag="wsb", name=f"wsb_{qb}")
            nc.scalar.copy(wsb[:], wacc[:])
            wT_view = wT_out[:].rearrange("(kt p) q -> p kt q", p=P)[:, :, q0:q0 + QB]
            nc.sync.dma_start(
                out=wT_view, in_=wsb[:].rearrange("p (kt q) -> p kt q", q=QB))
            # ---- out_proj + residual + LN (per 128-row q-subtile) ----
            attnT = state.pop(("attnT", qb))
            x_sb = state.pop(("x", qb))
            for qs in range(2):
                y_sb = ysb_pool.tile([P, E], BF16, tag="ysb", name=f"y_{qb}_{qs}")
                for eb in range(2):
                    ps = scps.tile([P, 512], F32, tag="sc", name=f"op_{qb}_{qs}_{eb}")
                    for c in range(NEC):
                        nc.tensor.matmul(
                            ps[:],
                            attnT[:, QB * c + P * qs: QB * c + P * (qs + 1)],
                            woT_sb[:, E * c + 512 * eb: E * c + 512 * (eb + 1)],
                            start=(c == 0), stop=False, skip_group_check=True,
                        )
                    nc.tensor.matmul(
                        ps[:], ones1[0:1, :],
                        bvbo_row[0:1, E + 512 * eb: E + 512 * (eb + 1)],
                        start=False, stop=True, skip_group_check=True,
                    )
                    nc.vector.tensor_tensor(
                        out=y_sb[:, 512 * eb:512 * (eb + 1)], in0=ps[:],
                        in1=x_sb[:, E * qs + 512 * eb: E * qs + 512 * (eb + 1)],
                        op=OP.add)
                stats = small.tile([P, 2, 6], F32, tag="stats", name=f"st_{qb}_{qs}")
                ysg = y_sb[:].rearrange("p (s f) -> p s f", f=512)
                for sg in range(2):
                    nc.vector.bn_stats(out=stats[:, sg, :], in_=ysg[:, sg, :])
                mv = small.tile([P, 2], F32, tag="mv", name=f"mv_{qb}_{qs}")
                nc.vector.bn_aggr(out=mv[:], in_=stats[:])
                lnv = small.tile([P, 1], F32, tag="lnv", name=f"lnv_{qb}_{qs}")
                nc.scalar.activation(lnv[:], mv[:, 1:2], AF.Ln, bias=eps_sb[:])
                rstd = small.tile([P, 1], F32, tag="rstd", name=f"rs_{qb}_{qs}")
                nc.scalar.activation(rstd[:], lnv[:], AF.Exp, scale=-0.5)
                nc.vector.tensor_scalar(
                    out=y_sb[:], in0=y_sb[:], scalar1=mv[:, 0:1], scalar2=rstd[:],
                    op0=OP.subtract, op1=OP.mult,
                )
                nc.vector.tensor_tensor(out=y_sb[:], in0=y_sb[:], in1=gamma_bc[:],
                                        op=OP.mult)
                nc.vector.tensor_tensor(out=y_sb[:], in0=y_sb[:], in1=beta_bc[:],
                                        op=OP.add)
                nc.sync.dma_start(out=y_out[q0 + P * qs:q0 + P * (qs + 1), :],
                                  in_=y_sb[:])

        # ---- pipelined emission ----
        emit_qb_head(0)
        for qb in range(NQB):
            if qb + 1 < NQB:
                emit_qb_head(qb + 1)
            for h in range(H):
                emit_scores(qb, h)
                if h >= 1:
                    emit_av(qb, h - 1)
                if h >= 2:
                    emit_identity(qb, h - 2)
                if h >= 1:
                    emit_bcast(qb, h - 1)
            # drain this q-block
            emit_av(qb, H - 1)
            emit_identity(qb, H - 2)
            emit_bcast(qb, H - 1)
            emit_identity(qb, H - 1)
            emit_qb_tail(qb)


_CACHED = None


def _build():
    global _CACHED
    if _CACHED is not None:
        return _CACHED
    nc = bacc.Bacc("TRN2", target_bir_lowering=False, debug=False, num_devices=8)
    io = {}
    for name in ["qT_in", "kT_in", "vT_in", "x_q", "wq", "wk", "wv", "wo"]:
        io[name] = nc.dram_tensor(name, [E, E], BF16, kind="ExternalInput").ap()
    io["b_all"] = nc.dram_tensor("b_all", [4, E], F32, kind="ExternalInput").ap()
    io["gb"] = nc.dram_tensor("gb", [2, E], F32, kind="ExternalInput").ap()
    io["y_out"] = nc.dram_tensor("y_out", [L, L], BF16, kind="ExternalOutput").ap()
    io["wT_out"] = nc.dram_tensor("wT_out", [L, L], BF16, kind="ExternalOutput").ap()
    with tile.TileContext(nc) as tc:
        with ExitStack() as ctx:
            tc.ctx = ctx
            _emit(nc, tc, io)
    nc.compile()
    _CACHED = nc
    return nc


def kernel(query, key_t, value, in_proj_w, in_proj_b, out_proj_w, out_proj_b,
           ln_gamma, ln_beta, _trace=False, _tmpdir=None):
    query = np.asarray(query, dtype=np.float32)
    key_t = np.asarray(key_t, dtype=np.float32)
    value = np.asarray(value, dtype=np.float32)
    qT = np.ascontiguousarray(query.transpose(0, 2, 1)).astype(BF16NP)
    kT = np.ascontiguousarray(key_t.transpose(0, 2, 1)).astype(BF16NP)
    vT = np.ascontiguousarray(value.transpose(0, 2, 1)).astype(BF16NP)
    x_q = query.astype(BF16NP)
    in_proj_w = np.asarray(in_proj_w, dtype=np.float32)
    wq = np.ascontiguousarray(in_proj_w[0:E].T).astype(BF16NP)
    wk = np.ascontiguousarray(in_proj_w[E:2 * E].T).astype(BF16NP)
    wv = np.ascontiguousarray(in_proj_w[2 * E:3 * E].T).astype(BF16NP)
    wo = np.ascontiguousarray(np.asarray(out_proj_w, dtype=np.float32).T).astype(BF16NP)
    b = np.asarray(in_proj_b, dtype=np.float32)
    b_all = np.ascontiguousarray(
        np.stack([b[0:E], b[E:2 * E], b[2 * E:3 * E],
                  np.asarray(out_proj_b, dtype=np.float32)])
    )
    gb = np.ascontiguousarray(
        np.stack([np.asarray(ln_gamma, dtype=np.float32),
                  np.asarray(ln_beta, dtype=np.float32)])
    )
    nc = _build()
    in_maps = [
        dict(qT_in=qT[c], kT_in=kT[c], vT_in=vT[c], x_q=x_q[c],
             wq=wq, wk=wk, wv=wv, wo=wo, b_all=b_all, gb=gb)
        for c in range(8)
    ]
    res = run_bass_kernel_spmd(
        nc, in_maps, core_ids=list(range(8)), trace=_trace, tmpdir=_tmpdir
    )
    y = np.stack([r["y_out"].astype(np.float32) for r in res.results])
    w = np.stack([r["wT_out"].T.astype(np.float32) for r in res.results])
    kernel._last_result = res
    return y, w
